# revision 39
# baseline (speedup 1.0000x reference)
"""MoE (top-2 of 16 routed experts + 2 shared experts) Trainium2 kernel.

Strategy: expert-parallel over 8 NeuronCores, with token dispatch done on
host (the router is 0.01% of the FLOPs; computing it host-side lets each
core receive exactly the tokens it needs, already gathered, transposed and
packed for single-descriptor-per-partition DMA).

Per core (SPMD program, identical on all cores; per-core in_maps differ):
  slot "r1": routed expert (the 8 least-loaded experts' slot, padded to cap)
  slot "r0": routed expert (8 most-loaded experts' slot, padded to cap)
  slot "sh": one shared expert applied to one quarter of the tokens (1024)
Each slot is a dense FFN in feature-major ("transposed") layout:
  mid^T[d,t] = gelu( sum_h Wup[h,d] * x^T[h,t] )   (fp32 psum)
  y^T[h,t]   =       sum_d Wdn[d,h] * mid^T[d,t]
Host scatter-adds y^T into the [T,H] output with the router gate weights
(gelu is the only nonlinearity, so the per-token gate commutes with the
down projection).

Precision: the shared experts run in bf16.  The routed experts run in
fp8-e4m3 with DoubleRow matmuls (2 contraction subtiles per instruction,
2x PE throughput): weights are pre-scaled by WS=32 so they sit in e4m3's
normal range, the gelu activation rescales by 1/WS on the way to the fp8
mid tensor, and the host folds the remaining 1/WS into the gate weights.
The routed path carries only ~19% of the output norm (gates are softmax
probs ~0.2), so its ~5% fp8 noise lands at ~1.1e-2 relative error overall
(vs the 2e-2 budget; measured 3.8e-3 in all-bf16).

Weights are packed column-major-of-tiles (wup d-major, wdn h-major) so the
first matmul chain only needs one 128-column weight slab + one x chunk,
and all DRAM tensors are [128, free] with per-partition contiguous data,
so every DMA moves large contiguous blocks per partition.
"""

import numpy as np
import ml_dtypes

import concourse.mybir as mybir
import concourse.tile as tile
from concourse import bacc
from concourse.bass_utils import run_bass_kernel_spmd

BF16 = mybir.dt.bfloat16
F8 = mybir.dt.float8e4
F32 = mybir.dt.float32
NP_BF16 = ml_dtypes.bfloat16
NP_F8 = ml_dtypes.float8_e4m3

B, S, H, D = 4, 1024, 2048, 1024
T = B * S
E_RT, E_SH, CORES = 16, 2, 8
HT, DT = H // 128, D // 128  # h-tiles (16), d-tiles (8)
N_SH = T // (CORES // E_SH)  # shared-slot tokens per core (1024)
TT = 512                     # max moving-operand / psum tile width
YG = 4                       # output h-tiles staged per store DMA
WS = 32.0                    # fp8 weight pre-scale
WARM = 8                     # PE warmup matmuls (cold-clock ~3.5us; the
                             # fixed ~7us engine-barrier preamble delays
                             # the first DMA, so slot-0 data lands ~10us)

_prog_cache = {}
LAST_RESULTS = None  # BassKernelResults of the most recent run (for test.py)


def _chunks(n):
    """Split n into the fewest near-equal chunks of width <= TT, each a
    multiple of 16 (fp8 DoubleRow needs 16B-aligned AP steps)."""
    assert n % 16 == 0
    k = -(-n // TT)
    out, off = [], 0
    for i in range(k, 0, -1):
        w = -(-(n - off) // i // 16) * 16
        out.append((off, w))
        off += w
    return out


def _build_program(r0_cap, r1_cap):
    nc = bacc.Bacc("TRN2", target_bir_lowering=False, debug=False,
                   num_devices=CORES)
    # Smallest slot first: its load is the only one nothing hides behind.
    # The big bf16 shared slot goes last; its weights live in a dedicated
    # single-buffer pool so their 8MB starts streaming right behind r0's
    # loads instead of waiting for r1's weight buffers to free.
    slots = []
    for name, n, dt in (("r1", r1_cap, F8), ("r0", r0_cap, F8),
                        ("sh", N_SH, BF16)):
        xd = nc.dram_tensor(f"x_{name}", [128, HT * n], dt,
                            kind="ExternalInput")
        wu = nc.dram_tensor(f"wup_{name}", [128, DT * HT * 128], dt,
                            kind="ExternalInput")
        wd = nc.dram_tensor(f"wdn_{name}", [128, HT * DT * 128], dt,
                            kind="ExternalInput")
        yd = nc.dram_tensor(f"y_{name}", [128, HT * n], BF16,
                            kind="ExternalOutput")
        slots.append((name, n, dt, xd, wu, wd, yd))

    with tile.TileContext(nc) as tc:
        with (
            tc.tile_pool(name="wpool", bufs=2) as wpool,
            tc.tile_pool(name="wshpool", bufs=1) as wshpool,
            tc.tile_pool(name="xpool", bufs=6) as xpool,
            tc.tile_pool(name="mpool", bufs=2) as mpool,
            tc.tile_pool(name="ypool", bufs=3) as ypool,
            tc.tile_pool(name="ps1pool", bufs=4, space="PSUM") as ps1pool,
            tc.tile_pool(name="ps2pool", bufs=4, space="PSUM") as ps2pool,
        ):
            # DMA orchestration.  Two HWDGE rings exist (issued via SP and
            # ACT); each processes its DMAs in issue order, and an issue
            # blocks while its destination pool slot is busy.  The sync
            # ring carries only loads (never compute), so prefetches for
            # slot s+1 are *emitted* before slot s's compute and stream
            # during it.  The ACT ring carries the y stores, plus half of
            # slot 0's loads (nothing to hide the first slot behind, so
            # both rings share its critical path).
            def emit_loads(si):
                name, n, dt, xd, wu, wd, yd = slots[si]
                wp = wshpool if dt == BF16 else wpool
                # wup SBUF layout [hp, dj, ht, dc]: one dj slab is the full
                # 128-col weight set for one up-proj accumulation chain.
                wut = wp.tile([128, DT, HT, 128], dt, tag="wup",
                              name=f"wup_{name}")
                # wdn SBUF layout [dp, hi, dj, hc].
                wdt = wp.tile([128, HT, DT, 128], dt, tag="wdn",
                              name=f"wdn_{name}")
                xts = []

                def load_x(ci, eng, halves=False):
                    # bf16 (shared) chunks split into two half-h tiles the
                    # same 8KB/partition size as the fp8 tiles, so the
                    # 6-deep x ring never alloc-blocks the sync ring while
                    # routed tiles are still live
                    off, w = _chunks(n)[ci]
                    hh = HT // 2
                    if dt == BF16:
                        pair = []
                        for h0 in (0, hh):
                            xt = xpool.tile([128, hh, w], dt, tag="x",
                                            name=f"x_{name}_{off}_{h0}")
                            eng.dma_start(
                                out=xt[:],
                                in_=xd[:, HT * off + h0 * w:
                                       HT * off + (h0 + hh) * w].rearrange(
                                    "p (h w) -> p h w", h=hh))
                            pair.append((xt, h0))
                        xts.append(pair)
                        return
                    xt = xpool.tile([128, HT, w], dt, tag="x",
                                    name=f"x_{name}_{off}")
                    xts.append([(xt, 0)])
                    pieces = ([(0, hh), (hh, HT - hh)] if halves
                              else [(0, HT)])
                    engs = eng if isinstance(eng, tuple) else (eng,)
                    for pi, (h0, hn) in enumerate(pieces):
                        engs[pi % len(engs)].dma_start(
                            out=xt[:, h0:h0 + hn, :],
                            in_=xd[:, HT * off + h0 * w:
                                   HT * off + (h0 + hn) * w].rearrange(
                                "p (h w) -> p h w", h=hn))

                def load_wup(gi, eng, dg=2):
                    g = gi * dg
                    eng.dma_start(
                        out=wut[:, g:g + dg, :, :],
                        in_=wu[:, g * HT * 128:(g + dg) * HT * 128]
                        .rearrange("p (c h w) -> p c h w", c=dg, h=HT))

                def load_wdn(gi, eng, hg=4):
                    g = gi * hg
                    eng.dma_start(
                        out=wdt[:, g:g + hg, :, :],
                        in_=wd[:, g * DT * 128:(g + hg) * DT * 128]
                        .rearrange("p (c d w) -> p c d w", c=hg, d=DT))

                if si == 0:
                    # first slot has nothing to hide behind: split its
                    # loads across both rings in consumption order
                    load_x(0, nc.sync, halves=True)
                    load_wup(0, nc.scalar)
                    load_wup(1, nc.sync)
                    load_wup(2, nc.scalar)
                    load_wup(3, nc.sync)
                    if len(_chunks(n)) > 1:
                        load_x(1, nc.scalar)
                    load_wdn(0, nc.sync)
                    load_wdn(1, nc.scalar)
                    load_wdn(2, nc.sync)
                    load_wdn(3, nc.scalar)
                else:
                    # later slots prefetch ONLY on the load-only sync
                    # ring.  DMA issues whose ring slot or pool buffer is
                    # busy block the whole issuing engine (strict FIFO) -
                    # on the ACT engine that wedges the gelu activations
                    # and starves the PE, so no loads ever go there.
                    load_x(0, nc.sync)
                    for gi in range(4):
                        load_wup(gi, nc.sync)
                    for gi in range(4):
                        load_wdn(gi, nc.sync)
                    for ci in range(1, len(_chunks(n))):
                        load_x(ci, nc.sync)
                return wut, wdt, xts

            # Dummy matmuls on scratch tiles: the PE HAM clock-gate only
            # lifts to 2.4 GHz after ~5us of sustained activity, so warm
            # it up while the first loads stream in.  (The memsets gate
            # the warmup on the vector engine's ~6us preamble - the tile
            # framework rejects reads of never-written tiles, so they
            # cannot be dropped.)
            wlhs = xpool.tile([128, 128], BF16, tag="warm_l", bufs=1,
                              name="warm_lhs")
            wrhs = xpool.tile([128, TT], BF16, tag="warm_r", bufs=1,
                              name="warm_rhs")
            nc.vector.memset(wlhs[:], 0)
            nc.vector.memset(wrhs[:], 0)
            wps = ps1pool.tile([128, TT], F32, tag="ps1", name="warm_ps")
            for wi in range(WARM):
                nc.tensor.matmul(wps[:], lhsT=wlhs[:], rhs=wrhs[:],
                                 start=True, stop=True)

            # all slots' loads are emitted upfront: ring order is emit
            # order, pool-slot alloc-waits pace the rings naturally, and
            # slot 2's ACT-ring pieces must precede the first y-store
            # issues in the ACT queue
            loaded = [emit_loads(0), emit_loads(1), emit_loads(2)]

            for si, (name, n, dt, xd, wu, wd, yd) in enumerate(slots):
                wut, wdt, xts = loaded[si]
                fp8 = dt == F8
                step = 2 if fp8 else 1
                pm = mybir.MatmulPerfMode.DoubleRow if fp8 else None

                for ci, (off, w) in enumerate(_chunks(n)):
                    base = HT * off
                    xpieces = xts[ci]

                    def xslice(hi):
                        for t, hb in reversed(xpieces):
                            if hi >= hb:
                                return (t[:, hi - hb:hi - hb + 2, :] if fp8
                                        else t[:, hi - hb, :])

                    mid = mpool.tile([128, DT, w], dt, tag="mid",
                                     name=f"mid_{name}_{off}")
                    for dj in range(DT):
                        ps = ps1pool.tile([128, TT], F32, tag="ps1",
                                          name=f"ps1_{name}_{off}_{dj}")
                        for hi in range(0, HT, step):
                            nc.tensor.matmul(
                                ps[:, :w],
                                lhsT=(wut[:, dj, hi:hi + 2, :] if fp8
                                      else wut[:, dj, hi, :]),
                                rhs=xslice(hi),
                                start=(hi == 0),
                                stop=(hi + step == HT),
                                perf_mode=pm,
                            )
                        # fp8: psum holds WS*pre_gelu; rescale before gelu
                        # and emit the mid tensor directly in fp8.
                        nc.scalar.activation(
                            mid[:, dj, :], ps[:, :w],
                            mybir.ActivationFunctionType.Gelu,
                            scale=(1.0 / WS if fp8 else 1.0))

                    for hi in range(HT):
                        ps2 = ps2pool.tile([128, TT], F32, tag="ps2",
                                           name=f"ps2_{name}_{off}_{hi}")
                        for dj in range(0, DT, step):
                            nc.tensor.matmul(
                                ps2[:, :w],
                                lhsT=(wdt[:, hi, dj:dj + 2, :] if fp8
                                      else wdt[:, hi, dj, :]),
                                rhs=(mid[:, dj:dj + 2, :] if fp8
                                     else mid[:, dj, :]),
                                start=(dj == 0),
                                stop=(dj + step == DT),
                                perf_mode=pm,
                            )
                        g = hi % YG
                        if g == 0:
                            yt = ypool.tile([128, YG, w], BF16, tag="y",
                                            name=f"y_{name}_{off}_{hi}")
                        nc.vector.tensor_copy(yt[:, g, :], ps2[:, :w])
                        if g == YG - 1:
                            # store on the ACT HWDGE ring (sync ring carries
                            # the loads)
                            lo = base + (hi - g) * w
                            nc.scalar.dma_start(
                                out=yd[:, lo:lo + YG * w].rearrange(
                                    "p (h w) -> p h w", h=YG),
                                in_=yt[:])
    nc.compile()
    return nc


def _pack_rows(a, nt):
    """[nt*128, m] row-major -> [128, nt*m] with per-partition contiguous
    (tile-major) layout."""
    m = a.shape[1]
    return np.ascontiguousarray(
        a.reshape(nt, 128, m).transpose(1, 0, 2).reshape(128, nt * m))


def _pack_x(xTc, npdt):
    """[H, n] -> [128, HT*n] chunk-major."""
    n = xTc.shape[1]
    parts = [_pack_rows(xTc[:, off:off + w], HT) for off, w in _chunks(n)]
    return np.ascontiguousarray(
        np.concatenate(parts, axis=1).astype(npdt))


def _pack_wup(w2d, npdt):
    """[H, D] -> [128, DT*HT*128] d-major ([hp, dj, ht, dc])."""
    return np.ascontiguousarray(
        w2d.reshape(HT, 128, DT, 128).transpose(1, 2, 0, 3)
        .reshape(128, DT * HT * 128).astype(npdt))


def _pack_wdn(w2d, npdt):
    """[D, H] -> [128, HT*DT*128] h-major ([dp, hi, dj, hc])."""
    return np.ascontiguousarray(
        w2d.reshape(DT, 128, HT, 128).transpose(1, 2, 0, 3)
        .reshape(128, HT * DT * 128).astype(npdt))


def _q8(a):
    """fp32 -> TRN e4m3 (max +-240)."""
    return np.clip(a, -240.0, 240.0).astype(NP_F8)


def _unpack_y(yflat, n):
    """[128, HT*n] chunk-major -> [n, H] (token-major)."""
    yflat = yflat.astype(np.float32)
    out = np.empty((n, H), np.float32)
    base = 0
    for off, w in _chunks(n):
        blk = yflat[:, base:base + HT * w].reshape(128, HT, w)
        out[off:off + w] = blk.transpose(2, 1, 0).reshape(w, H)
        base += HT * w
    return out


def _route(x2d, w_router):
    """Top-2 routing, matching the reference's softmax-then-top_k."""
    logits = x2d @ w_router
    m = logits.max(-1, keepdims=True)
    e = np.exp(logits - m)
    probs = e / e.sum(-1, keepdims=True)
    rows = np.arange(x2d.shape[0])
    i1 = probs.argmax(-1)
    masked = probs.copy()
    masked[rows, i1] = -np.inf
    i2 = masked.argmax(-1)
    return probs, i1, i2


def kernel(x, Wsh_up, Wsh_down, Wrt_up, Wrt_down, W_router):
    global LAST_RESULTS
    x = np.asarray(x, np.float32)
    Wsh_up = np.asarray(Wsh_up, np.float32)
    Wsh_down = np.asarray(Wsh_down, np.float32)
    Wrt_up = np.asarray(Wrt_up, np.float32)
    Wrt_down = np.asarray(Wrt_down, np.float32)
    W_router = np.asarray(W_router, np.float32)

    x2d = x.reshape(T, H)
    probs, i1, i2 = _route(x2d, W_router)

    # token ids / gate values per routed expert
    ids, gates = [], []
    for e in range(E_RT):
        sel = np.where((i1 == e) | (i2 == e))[0]
        ids.append(sel)
        gates.append(probs[sel, e].astype(np.float32))

    # slot r0 takes the 8 most-loaded experts, r1 the 8 least-loaded, so
    # the two static capacities hug the actual counts.
    order = sorted(range(E_RT), key=lambda e: -len(ids[e]))
    slot_experts = {0: order[:CORES], 1: order[CORES:]}
    caps = []
    for slot in range(2):
        mx = max(len(ids[e]) for e in slot_experts[slot])
        caps.append(max(512, -(-mx // 32) * 32))
    r0_cap, r1_cap = caps

    key = (r0_cap, r1_cap)
    if key not in _prog_cache:
        _prog_cache[key] = _build_program(r0_cap, r1_cap)
    nc = _prog_cache[key]

    in_maps = []
    for c in range(CORES):
        se, q = c % E_SH, c // E_SH
        m = {
            "x_sh": _pack_x(np.ascontiguousarray(
                x2d[q * N_SH:(q + 1) * N_SH].T), NP_BF16),
            "wup_sh": _pack_wup(Wsh_up[se], NP_BF16),
            "wdn_sh": _pack_wdn(Wsh_down[se], NP_BF16),
        }
        for slot, cap in ((0, r0_cap), (1, r1_cap)):
            e = slot_experts[slot][c]
            sel = ids[e]
            xe = np.zeros((H, cap), np.float32)
            xe[:, :len(sel)] = x2d[sel].T
            m[f"x_r{slot}"] = _pack_x(xe, NP_F8)
            m[f"wup_r{slot}"] = _pack_wup(_q8(Wrt_up[e] * WS), NP_F8)
            m[f"wdn_r{slot}"] = _pack_wdn(_q8(Wrt_down[e] * WS), NP_F8)
        in_maps.append(m)

    res = run_bass_kernel_spmd(nc, in_maps, core_ids=list(range(CORES)))
    LAST_RESULTS = res

    out = np.zeros((T, H), np.float32)
    for c in range(CORES):
        q = c // E_SH
        out[q * N_SH:(q + 1) * N_SH] += _unpack_y(res.results[c]["y_sh"],
                                                  N_SH)
    for slot, cap in ((0, r0_cap), (1, r1_cap)):
        for c in range(CORES):
            e = slot_experts[slot][c]
            sel = ids[e]
            y = _unpack_y(res.results[c][f"y_r{slot}"], cap)
            # fp8 path: one WS factor (down-proj weights) still scales y
            out[sel] += (gates[e] / WS)[:, None] * y[:len(sel)]
    return out.reshape(B, S, H)
